# revision 25
# baseline (speedup 1.0000x reference)
"""Trainium2 Bass kernel for nn_Net_89343909691628 (moe_routing).

Reference computation (per row b of x):
    xf   = x.reshape(B, -1)[:, :1024]            (= channel 0 of x)
    e    = relu(xf @ W_embed.T + b_embed)        [B, 16]
    gate = softmax(e @ Wg.T + bg)                [B, 4]
    h    = relu(einsum('bd,edh', e, W1) + b1)    [B, 4, 16]
    eo   = einsum('beh,eho', h, W2) + b2         [B, 4, 1000]
    out  = einsum('beo,be->bo', eo, gate)        [B, 1000]

Kernel algebra: everything after the exp is linear in the (unnormalized)
gate, so with g = exp(logits) and den = sum_e g_e:
    out = (concat_e[g_e * h_e] @ concat_e[W2_e] + g @ b2) / den
which is a single [B,68] @ [68,1000] matmul (4 gate rows appended to carry
the b2 term) followed by one per-row scale by 1/den.

Sharding: pure data parallel over 8 cores, 4096 rows per core. All on-chip
work is kept in "feature-major" [feat, batch] orientation so no activation
transposes are needed; x is pre-transposed/tiled on the host.
"""

import os
import numpy as np

B = 32768
NCORES = 8
BL = B // NCORES          # 4096 rows per core
D = 1024                  # used input features (= 32*32, channel 0)
J = 16                    # embed dim
NE = 4                    # experts
T = 16                    # expert hidden
H = NE * T                # 64
HG = H + NE               # 68 = h rows + gate rows
OUT = 1000
SUP = 512                 # rows per super-tile
NSUP = BL // SUP          # 8
PT = 128                  # partitions
NC_CHUNK = D // PT        # 8 contraction chunks for the embed matmul

LAST_RESULTS = None       # BassKernelResults of the most recent run (for profiling)

# "f32r"  : fp32 data, fp32r matmuls (~2e-4 l2 rel err)
# "bf16in": x / W_embed in bf16, rest fp32r (~1.3e-3)
# "bf16"  : all matmul operands bf16 (~2e-3), fastest
PRECISION = os.environ.get("KERNEL_PRECISION", "f32r")

_BUILT = None


def _build():
    """Build the Bass module (one NeuronCore program, run SPMD on 8 cores)."""
    global _BUILT
    if _BUILT is not None:
        return _BUILT

    from contextlib import ExitStack

    import concourse.mybir as mybir
    import concourse.tile as tile
    from concourse import bacc

    fp32 = mybir.dt.float32
    f32r = mybir.dt.float32r
    bf16 = mybir.dt.bfloat16
    xdt = bf16 if PRECISION in ("bf16in", "bf16") else f32r
    mdt = bf16 if PRECISION == "bf16" else f32r

    nc = bacc.Bacc(trn_type="TRN2", target_bir_lowering=False, debug=False)

    xt_d = nc.dram_tensor("xt", [NSUP, PT, NC_CHUNK, SUP], xdt, kind="ExternalInput").ap()
    wembt_d = nc.dram_tensor("wembt", [PT, NC_CHUNK, J], xdt, kind="ExternalInput").ap()
    bembt_d = nc.dram_tensor("bembt", [J, 1], fp32, kind="ExternalInput").ap()
    w1wg_d = nc.dram_tensor("w1wg", [J, HG], mdt, kind="ExternalInput").ap()
    b1bg_d = nc.dram_tensor("b1bg", [HG, 1], fp32, kind="ExternalInput").ap()
    eo4_d = nc.dram_tensor("eo4", [NE, H], mdt, kind="ExternalInput").ap()
    w2aug_d = nc.dram_tensor("w2aug", [HG, OUT + 2], mdt, kind="ExternalInput").ap()
    out_d = nc.dram_tensor("out_t", [NSUP, PT, SUP // PT, OUT], fp32, kind="ExternalOutput").ap()

    KT = SUP // PT  # 4 row-tiles of 128 per super-tile

    with tile.TileContext(nc) as tc, ExitStack() as ctx:
        consts = ctx.enter_context(tc.tile_pool(name="consts", bufs=1))
        xpool = ctx.enter_context(tc.tile_pool(name="xpool", bufs=4))
        epool = ctx.enter_context(tc.tile_pool(name="epool", bufs=3))
        hpool = ctx.enter_context(tc.tile_pool(name="hpool", bufs=3))
        spool = ctx.enter_context(tc.tile_pool(name="spool", bufs=3))
        opool = ctx.enter_context(tc.tile_pool(name="opool", bufs=3))
        pe_e = ctx.enter_context(tc.tile_pool(name="pe_e", bufs=1, space="PSUM"))
        pe_hg = ctx.enter_context(tc.tile_pool(name="pe_hg", bufs=2, space="PSUM"))
        pe_rep = ctx.enter_context(tc.tile_pool(name="pe_rep", bufs=1, space="PSUM"))
        pe_o = ctx.enter_context(tc.tile_pool(name="pe_o", bufs=2, space="PSUM"))

        # ---- persistent constants ----
        wembt_s = consts.tile([PT, NC_CHUNK, J], xdt)
        nc.scalar.dma_start(out=wembt_s, in_=wembt_d)
        bembt_s = consts.tile([J, 1], fp32)
        nc.scalar.dma_start(out=bembt_s, in_=bembt_d)
        w1wg_s = consts.tile([J, HG], mdt)
        nc.scalar.dma_start(out=w1wg_s, in_=w1wg_d)
        b1bg_s = consts.tile([HG, 1], fp32)
        nc.scalar.dma_start(out=b1bg_s, in_=b1bg_d)
        w2aug_s = consts.tile([HG, OUT + 2], mdt)
        nc.scalar.dma_start(out=w2aug_s, in_=w2aug_d)
        # expert-replication matrix (rows 64:68 so base partitions line up
        # with the gate rows of hgsT)
        eo4_s = consts.tile([H + NE, H], mdt)
        nc.scalar.dma_start(out=eo4_s[H:H + NE, :], in_=eo4_d)
        def head(s):
            """Load x(s), run the embed matmuls + relu -> eT_s."""
            xt = xpool.tile([PT, NC_CHUNK, SUP], xdt, name=f"xt{s}", tag="xt")
            hc = NC_CHUNK // 2
            nc.sync.dma_start(out=xt[:, 0:hc, :], in_=xt_d[s][:, 0:hc, :])
            # the first two loads may also use the scalar HWDGE row: ACT has
            # no compute queued yet, so no sequencer-stall hazard, and the
            # read-only startup phase gets both rings
            eng_in = nc.scalar if s < 2 else nc.sync
            eng_in.dma_start(out=xt[:, hc:, :], in_=xt_d[s][:, hc:, :])

            # embed: eT[j, b] = sum_d WembT[d, j] * xT[d, b]
            eT_p = pe_e.tile([J, SUP], fp32, name=f"eT_p{s}", tag="eT_p")
            for c in range(NC_CHUNK):
                nc.tensor.matmul(
                    eT_p, lhsT=wembt_s[:, c, :], rhs=xt[:, c, :],
                    start=(c == 0), stop=(c == NC_CHUNK - 1),
                )
            eT_s = epool.tile([J, SUP], mdt, name=f"eT_s{s}", tag="eT_s")
            nc.scalar.activation(
                out=eT_s, in_=eT_p,
                func=mybir.ActivationFunctionType.Relu, bias=bembt_s, scale=1.0,
            )
            return eT_s

        def tail(s, eT_s):
            """Experts fc1+gate, gate scaling, fc2, final scale, store."""
            # experts fc1 + gate logits, fused: hgT = [W1cat | WgT].T @ e
            hgT_p = pe_hg.tile([HG, SUP], fp32, name=f"hgT_p{s}", tag="hgT_p")
            nc.tensor.matmul(hgT_p, lhsT=w1wg_s, rhs=eT_s, start=True, stop=True)

            hgsT = hpool.tile([HG, SUP], mdt, name=f"hgsT{s}", tag="hgsT")
            # gate rows: unnormalized softmax numerators (exp of logits+bg)
            nc.scalar.activation(
                out=hgsT[H:HG, :], in_=hgT_p[H:HG, :],
                func=mybir.ActivationFunctionType.Exp, bias=b1bg_s[H:HG, :], scale=1.0,
            )
            # h rows: relu(fc1 + b1)
            hrelu = hpool.tile([H, SUP], fp32, name=f"hrelu{s}", tag="hrelu")
            nc.scalar.activation(
                out=hrelu, in_=hgT_p[0:H, :],
                func=mybir.ActivationFunctionType.Relu, bias=b1bg_s[0:H, :], scale=1.0,
            )

            # replicate gates over each expert's 16 rows:
            # rep[q, b] = gexp[q//16, b]
            rep_p = pe_rep.tile([H, SUP], fp32, name=f"rep_p{s}", tag="rep_p")
            nc.tensor.matmul(
                rep_p, lhsT=eo4_s[H:H + NE, :], rhs=hgsT[H:HG, :],
                start=True, stop=True,
            )
            # gate-scaled h rows
            nc.vector.tensor_mul(hgsT[0:H, :], hrelu, rep_p[0:H, :])

            # fc2 + bias-through-gates + final 1/den scale.
            # w2aug's extra column is [0]*64+[1]*4, so the second matmul also
            # produces den = sum_e gexp per row, already per-partition.
            out_sb = opool.tile([PT, KT, OUT], fp32, name=f"out_sb{s}", tag="out_sb")
            for k in range(KT):
                o_p = pe_o.tile([PT, 2, 512], fp32, name=f"o_p{s}_{k}", tag="o_p")
                nc.tensor.matmul(
                    o_p[:, 0, 0:512], lhsT=hgsT[:, k * PT:(k + 1) * PT],
                    rhs=w2aug_s[:, 0:512], start=True, stop=True,
                )
                nc.tensor.matmul(
                    o_p[:, 1, 0:OUT + 2 - 512], lhsT=hgsT[:, k * PT:(k + 1) * PT],
                    rhs=w2aug_s[:, 512:OUT + 2], start=True, stop=True,
                )
                rden = spool.tile([PT, 1], fp32, name=f"rden{s}_{k}", tag="rden")
                nc.vector.reciprocal(rden, o_p[:, 1, OUT - 512:OUT + 1 - 512])
                nc.vector.tensor_scalar_mul(
                    out_sb[:, k, 0:512], o_p[:, 0, 0:512], rden
                )
                nc.vector.tensor_scalar_mul(
                    out_sb[:, k, 512:OUT], o_p[:, 1, 0:OUT - 512], rden
                )
                # late stores may also use the sync HWDGE row: the input
                # stream is finished by then, so the write-only drain phase
                # gets two rings
                eng_out = nc.sync if (s >= NSUP - 2 and k % 2 == 1) else nc.gpsimd
                eng_out.dma_start(out=out_d[s][:, k, :], in_=out_sb[:, k, :])

        # Software-pipelined emission: embed(s+1) is emitted before tail(s),
        # so the in-order PE stream never stalls waiting for the ACT/DVE
        # results that tail(s) consumes - they were produced a stage earlier.
        prev = None
        for s in range(NSUP):
            e = head(s)
            if prev is not None:
                tail(s - 1, prev)
            prev = e
        tail(NSUP - 1, prev)


    nc.compile()
    _BUILT = nc
    return nc


def _prep_host(inputs):
    """Shard + retile the full inputs into per-core in_maps."""
    import ml_dtypes
    xdt = ml_dtypes.bfloat16 if PRECISION in ("bf16in", "bf16") else np.float32
    mdt = ml_dtypes.bfloat16 if PRECISION == "bf16" else np.float32
    x = np.ascontiguousarray(np.asarray(inputs["x"], dtype=np.float32))
    W_embed = np.asarray(inputs["W_embed"], dtype=np.float32)
    b_embed = np.asarray(inputs["b_embed"], dtype=np.float32)
    W1 = np.asarray(inputs["W1"], dtype=np.float32)
    b1 = np.asarray(inputs["b1"], dtype=np.float32)
    W2 = np.asarray(inputs["W2"], dtype=np.float32)
    b2 = np.asarray(inputs["b2"], dtype=np.float32)
    Wg = np.asarray(inputs["Wg"], dtype=np.float32)
    bg = np.asarray(inputs["bg"], dtype=np.float32)

    xf = x.reshape(B, -1)[:, :D]                       # [B, 1024] (channel 0)

    wembt = np.ascontiguousarray(
        W_embed.T.reshape(NC_CHUNK, PT, J).transpose(1, 0, 2)
    )                                                  # [128, 8, 16]
    bembt = np.ascontiguousarray(b_embed.reshape(J, 1))
    w1wg = np.ascontiguousarray(
        np.concatenate([W1.transpose(1, 0, 2).reshape(J, H), Wg.T], axis=1)
    )                                                  # [16, 68]
    b1bg = np.ascontiguousarray(
        np.concatenate([b1.reshape(H), bg]).reshape(HG, 1)
    )
    eo4 = np.zeros((NE, H), np.float32)
    for e in range(NE):
        eo4[e, e * T:(e + 1) * T] = 1.0
    dencols = np.zeros((HG, 2), np.float32)
    dencols[H:, 0] = 1.0     # den column; second column is zero padding so
    w2aug = np.ascontiguousarray(np.concatenate(      # the moving dim stays even
        [np.concatenate([W2.reshape(H, OUT), b2], axis=0), dencols], axis=1,
    ))                                                 # [68, 1002]

    shared = dict(wembt=wembt.astype(xdt), bembt=bembt,
                  w1wg=w1wg.astype(mdt), b1bg=b1bg, eo4=eo4.astype(mdt),
                  w2aug=w2aug.astype(mdt))
    in_maps = []
    for c in range(NCORES):
        xc = xf[c * BL:(c + 1) * BL]                   # [4096, 1024]
        # [s, p, chunk, b2] with row = s*512+b2, feature = chunk*128+p
        xt = np.ascontiguousarray(
            xc.reshape(NSUP, SUP, NC_CHUNK, PT).transpose(0, 3, 2, 1)
        ).astype(xdt)
        in_maps.append(dict(xt=xt, **shared))
    return in_maps


def kernel(**inputs):
    global LAST_RESULTS
    from concourse.bass_utils import run_bass_kernel_spmd

    nc = _build()
    in_maps = _prep_host(inputs)
    results = None
    last_exc = None
    for attempt in range(3):
        try:
            results = run_bass_kernel_spmd(
                nc, in_maps, core_ids=list(range(NCORES)),
                trace=bool(os.environ.get("KERNEL_TRACE")),
            )
            break
        except Exception as exc:  # transient NRT device wedge: reset + retry
            last_exc = exc
            try:
                import jax
                jax.clear_caches()
                jax.clear_backends()
            except Exception:
                pass
    if results is None:
        raise last_exc
    LAST_RESULTS = results

    outs = []
    for rmap in results.results:
        ot = rmap["out_t"]                             # [8, 128, 4, 1000]
        outs.append(ot.transpose(0, 2, 1, 3).reshape(BL, OUT))
    return np.ascontiguousarray(np.concatenate(outs, axis=0))


# revision 26
# speedup vs baseline: 1.3251x; 1.3251x over previous
"""Trainium2 Bass kernel for nn_Net_89343909691628 (moe_routing).

Reference computation (per row b of x):
    xf   = x.reshape(B, -1)[:, :1024]            (= channel 0 of x)
    e    = relu(xf @ W_embed.T + b_embed)        [B, 16]
    gate = softmax(e @ Wg.T + bg)                [B, 4]
    h    = relu(einsum('bd,edh', e, W1) + b1)    [B, 4, 16]
    eo   = einsum('beh,eho', h, W2) + b2         [B, 4, 1000]
    out  = einsum('beo,be->bo', eo, gate)        [B, 1000]

Kernel algebra: everything after the exp is linear in the (unnormalized)
gate, so with g = exp(logits) and den = sum_e g_e:
    out = (concat_e[g_e * h_e] @ concat_e[W2_e] + g @ b2) / den
which is a single [B,68] @ [68,1000] matmul (4 gate rows appended to carry
the b2 term) followed by one per-row scale by 1/den.

Sharding: pure data parallel over 8 cores, 4096 rows per core. All on-chip
work is kept in "feature-major" [feat, batch] orientation so no activation
transposes are needed; x is pre-transposed/tiled on the host.
"""

import os
import numpy as np

B = 32768
NCORES = 8
BL = B // NCORES          # 4096 rows per core
D = 1024                  # used input features (= 32*32, channel 0)
J = 16                    # embed dim
NE = 4                    # experts
T = 16                    # expert hidden
H = NE * T                # 64
HG = H + NE               # 68 = h rows + gate rows
OUT = 1000
SUP = 512                 # rows per super-tile
NSUP = BL // SUP          # 8
PT = 128                  # partitions
NC_CHUNK = D // PT        # 8 contraction chunks for the embed matmul

LAST_RESULTS = None       # BassKernelResults of the most recent run (for profiling)

# "f32r"  : fp32 data, fp32r matmuls (~2e-4 l2 rel err)
# "bf16in": x / W_embed in bf16, rest fp32r (~1.3e-3)
# "bf16"  : all matmul operands bf16 (~2e-3), fastest
PRECISION = os.environ.get("KERNEL_PRECISION", "f32r")

_BUILT = None


def _build():
    """Build the Bass module (one NeuronCore program, run SPMD on 8 cores)."""
    global _BUILT
    if _BUILT is not None:
        return _BUILT

    from contextlib import ExitStack

    import concourse.mybir as mybir
    import concourse.tile as tile
    from concourse import bacc

    fp32 = mybir.dt.float32
    f32r = mybir.dt.float32r
    bf16 = mybir.dt.bfloat16
    xdt = bf16 if PRECISION in ("bf16in", "bf16") else f32r
    mdt = bf16 if PRECISION == "bf16" else f32r

    nc = bacc.Bacc(trn_type="TRN2", target_bir_lowering=False, debug=False)

    xt_d = nc.dram_tensor("xt", [NSUP, PT, NC_CHUNK, SUP], xdt, kind="ExternalInput").ap()
    wembt_d = nc.dram_tensor("wembt", [PT, NC_CHUNK, J], xdt, kind="ExternalInput").ap()
    bembt_d = nc.dram_tensor("bembt", [J, 1], fp32, kind="ExternalInput").ap()
    w1wg_d = nc.dram_tensor("w1wg", [J, HG], mdt, kind="ExternalInput").ap()
    b1bg_d = nc.dram_tensor("b1bg", [HG, 1], fp32, kind="ExternalInput").ap()
    eo4_d = nc.dram_tensor("eo4", [NE, H], mdt, kind="ExternalInput").ap()
    w2aug_d = nc.dram_tensor("w2aug", [HG, OUT + 2], mdt, kind="ExternalInput").ap()
    out_d = nc.dram_tensor("out_t", [NSUP, PT, SUP // PT, OUT], fp32, kind="ExternalOutput").ap()

    KT = SUP // PT  # 4 row-tiles of 128 per super-tile

    with tile.TileContext(nc) as tc, ExitStack() as ctx:
        consts = ctx.enter_context(tc.tile_pool(name="consts", bufs=1))
        xpool = ctx.enter_context(tc.tile_pool(name="xpool", bufs=4))
        epool = ctx.enter_context(tc.tile_pool(name="epool", bufs=3))
        hpool = ctx.enter_context(tc.tile_pool(name="hpool", bufs=3))
        spool = ctx.enter_context(tc.tile_pool(name="spool", bufs=3))
        opool = ctx.enter_context(tc.tile_pool(name="opool", bufs=3))
        pe_e = ctx.enter_context(tc.tile_pool(name="pe_e", bufs=1, space="PSUM"))
        pe_hg = ctx.enter_context(tc.tile_pool(name="pe_hg", bufs=2, space="PSUM"))
        pe_rep = ctx.enter_context(tc.tile_pool(name="pe_rep", bufs=1, space="PSUM"))
        pe_o = ctx.enter_context(tc.tile_pool(name="pe_o", bufs=2, space="PSUM"))

        # ---- persistent constants ----
        wembt_s = consts.tile([PT, NC_CHUNK, J], xdt)
        nc.scalar.dma_start(out=wembt_s, in_=wembt_d)
        bembt_s = consts.tile([J, 1], fp32)
        nc.scalar.dma_start(out=bembt_s, in_=bembt_d)
        w1wg_s = consts.tile([J, HG], mdt)
        nc.scalar.dma_start(out=w1wg_s, in_=w1wg_d)
        b1bg_s = consts.tile([HG, 1], fp32)
        nc.scalar.dma_start(out=b1bg_s, in_=b1bg_d)
        w2aug_s = consts.tile([HG, OUT + 2], mdt)
        nc.scalar.dma_start(out=w2aug_s, in_=w2aug_d)
        # expert-replication matrix (rows 64:68 so base partitions line up
        # with the gate rows of hgsT)
        eo4_s = consts.tile([H + NE, H], mdt)
        nc.scalar.dma_start(out=eo4_s[H:H + NE, :], in_=eo4_d)
        def head(s):
            """Load x(s), run the embed matmuls + relu -> eT_s."""
            xt = xpool.tile([PT, NC_CHUNK, SUP], xdt, name=f"xt{s}", tag="xt")
            hc = NC_CHUNK // 2
            nc.sync.dma_start(out=xt[:, 0:hc, :], in_=xt_d[s][:, 0:hc, :])
            nc.sync.dma_start(out=xt[:, hc:, :], in_=xt_d[s][:, hc:, :])

            # embed: eT[j, b] = sum_d WembT[d, j] * xT[d, b]
            eT_p = pe_e.tile([J, SUP], fp32, name=f"eT_p{s}", tag="eT_p")
            for c in range(NC_CHUNK):
                nc.tensor.matmul(
                    eT_p, lhsT=wembt_s[:, c, :], rhs=xt[:, c, :],
                    start=(c == 0), stop=(c == NC_CHUNK - 1),
                )
            eT_s = epool.tile([J, SUP], mdt, name=f"eT_s{s}", tag="eT_s")
            nc.scalar.activation(
                out=eT_s, in_=eT_p,
                func=mybir.ActivationFunctionType.Relu, bias=bembt_s, scale=1.0,
            )
            return eT_s

        def tail(s, eT_s):
            """Experts fc1+gate, gate scaling, fc2, final scale, store."""
            # experts fc1 + gate logits, fused: hgT = [W1cat | WgT].T @ e
            hgT_p = pe_hg.tile([HG, SUP], fp32, name=f"hgT_p{s}", tag="hgT_p")
            nc.tensor.matmul(hgT_p, lhsT=w1wg_s, rhs=eT_s, start=True, stop=True)

            hgsT = hpool.tile([HG, SUP], mdt, name=f"hgsT{s}", tag="hgsT")
            # gate rows: unnormalized softmax numerators (exp of logits+bg)
            nc.scalar.activation(
                out=hgsT[H:HG, :], in_=hgT_p[H:HG, :],
                func=mybir.ActivationFunctionType.Exp, bias=b1bg_s[H:HG, :], scale=1.0,
            )
            # h rows: relu(fc1 + b1)
            hrelu = hpool.tile([H, SUP], fp32, name=f"hrelu{s}", tag="hrelu")
            nc.scalar.activation(
                out=hrelu, in_=hgT_p[0:H, :],
                func=mybir.ActivationFunctionType.Relu, bias=b1bg_s[0:H, :], scale=1.0,
            )

            # replicate gates over each expert's 16 rows:
            # rep[q, b] = gexp[q//16, b]
            rep_p = pe_rep.tile([H, SUP], fp32, name=f"rep_p{s}", tag="rep_p")
            nc.tensor.matmul(
                rep_p, lhsT=eo4_s[H:H + NE, :], rhs=hgsT[H:HG, :],
                start=True, stop=True,
            )
            # gate-scaled h rows
            nc.vector.tensor_mul(hgsT[0:H, :], hrelu, rep_p[0:H, :])

            # fc2 + bias-through-gates + final 1/den scale.
            # w2aug's extra column is [0]*64+[1]*4, so the second matmul also
            # produces den = sum_e gexp per row, already per-partition.
            out_sb = opool.tile([PT, KT, OUT], fp32, name=f"out_sb{s}", tag="out_sb")
            for k in range(KT):
                o_p = pe_o.tile([PT, 2, 512], fp32, name=f"o_p{s}_{k}", tag="o_p")
                nc.tensor.matmul(
                    o_p[:, 0, 0:512], lhsT=hgsT[:, k * PT:(k + 1) * PT],
                    rhs=w2aug_s[:, 0:512], start=True, stop=True,
                )
                nc.tensor.matmul(
                    o_p[:, 1, 0:OUT + 2 - 512], lhsT=hgsT[:, k * PT:(k + 1) * PT],
                    rhs=w2aug_s[:, 512:OUT + 2], start=True, stop=True,
                )
                rden = spool.tile([PT, 1], fp32, name=f"rden{s}_{k}", tag="rden")
                nc.vector.reciprocal(rden, o_p[:, 1, OUT - 512:OUT + 1 - 512])
                nc.vector.tensor_scalar_mul(
                    out_sb[:, k, 0:512], o_p[:, 0, 0:512], rden
                )
                nc.vector.tensor_scalar_mul(
                    out_sb[:, k, 512:OUT], o_p[:, 1, 0:OUT - 512], rden
                )
                nc.gpsimd.dma_start(out=out_d[s][:, k, :], in_=out_sb[:, k, :])

        # Software-pipelined emission: embed(s+1) is emitted before tail(s),
        # so the in-order PE stream never stalls waiting for the ACT/DVE
        # results that tail(s) consumes - they were produced a stage earlier.
        prev = None
        for s in range(NSUP):
            e = head(s)
            if prev is not None:
                tail(s - 1, prev)
            prev = e
        tail(NSUP - 1, prev)


    nc.compile()
    _BUILT = nc
    return nc


def _prep_host(inputs):
    """Shard + retile the full inputs into per-core in_maps."""
    import ml_dtypes
    xdt = ml_dtypes.bfloat16 if PRECISION in ("bf16in", "bf16") else np.float32
    mdt = ml_dtypes.bfloat16 if PRECISION == "bf16" else np.float32
    x = np.ascontiguousarray(np.asarray(inputs["x"], dtype=np.float32))
    W_embed = np.asarray(inputs["W_embed"], dtype=np.float32)
    b_embed = np.asarray(inputs["b_embed"], dtype=np.float32)
    W1 = np.asarray(inputs["W1"], dtype=np.float32)
    b1 = np.asarray(inputs["b1"], dtype=np.float32)
    W2 = np.asarray(inputs["W2"], dtype=np.float32)
    b2 = np.asarray(inputs["b2"], dtype=np.float32)
    Wg = np.asarray(inputs["Wg"], dtype=np.float32)
    bg = np.asarray(inputs["bg"], dtype=np.float32)

    xf = x.reshape(B, -1)[:, :D]                       # [B, 1024] (channel 0)

    wembt = np.ascontiguousarray(
        W_embed.T.reshape(NC_CHUNK, PT, J).transpose(1, 0, 2)
    )                                                  # [128, 8, 16]
    bembt = np.ascontiguousarray(b_embed.reshape(J, 1))
    w1wg = np.ascontiguousarray(
        np.concatenate([W1.transpose(1, 0, 2).reshape(J, H), Wg.T], axis=1)
    )                                                  # [16, 68]
    b1bg = np.ascontiguousarray(
        np.concatenate([b1.reshape(H), bg]).reshape(HG, 1)
    )
    eo4 = np.zeros((NE, H), np.float32)
    for e in range(NE):
        eo4[e, e * T:(e + 1) * T] = 1.0
    dencols = np.zeros((HG, 2), np.float32)
    dencols[H:, 0] = 1.0     # den column; second column is zero padding so
    w2aug = np.ascontiguousarray(np.concatenate(      # the moving dim stays even
        [np.concatenate([W2.reshape(H, OUT), b2], axis=0), dencols], axis=1,
    ))                                                 # [68, 1002]

    shared = dict(wembt=wembt.astype(xdt), bembt=bembt,
                  w1wg=w1wg.astype(mdt), b1bg=b1bg, eo4=eo4.astype(mdt),
                  w2aug=w2aug.astype(mdt))
    in_maps = []
    for c in range(NCORES):
        xc = xf[c * BL:(c + 1) * BL]                   # [4096, 1024]
        # [s, p, chunk, b2] with row = s*512+b2, feature = chunk*128+p
        xt = np.ascontiguousarray(
            xc.reshape(NSUP, SUP, NC_CHUNK, PT).transpose(0, 3, 2, 1)
        ).astype(xdt)
        in_maps.append(dict(xt=xt, **shared))
    return in_maps


def kernel(**inputs):
    global LAST_RESULTS
    from concourse.bass_utils import run_bass_kernel_spmd

    nc = _build()
    in_maps = _prep_host(inputs)
    results = None
    last_exc = None
    for attempt in range(3):
        try:
            results = run_bass_kernel_spmd(
                nc, in_maps, core_ids=list(range(NCORES)),
                trace=bool(os.environ.get("KERNEL_TRACE")),
            )
            break
        except Exception as exc:  # transient NRT device wedge: reset + retry
            last_exc = exc
            try:
                import jax
                jax.clear_caches()
                jax.clear_backends()
            except Exception:
                pass
    if results is None:
        raise last_exc
    LAST_RESULTS = results

    outs = []
    for rmap in results.results:
        ot = rmap["out_t"]                             # [8, 128, 4, 1000]
        outs.append(ot.transpose(0, 2, 1, 3).reshape(BL, OUT))
    return np.ascontiguousarray(np.concatenate(outs, axis=0))
